# revision 1
# baseline (speedup 1.0000x reference)
"""Trainium2 Bass kernel for nn_CompressedSensingConvolutional (hw-loop version).

Problem: 200 FISTA iterations of
    re    = conv_transpose(x - conv(y_tmp, w_conv, stride 8, SAME) - b_conv,
                           w_ct, stride 8, SAME) + b_ct
    w     = y_tmp - re
    y_new = soft_threshold(w, lam)        (per-sample lam)
    y_tmp = y_new + beta_n (y_new - y_last)
with x: (64,9,9,3), output y_new: (64,72,72,3).

Structure exploited (all exact, no approximations):
  * conv_transpose with 5x5 kernel / stride 8 writes NON-overlapping 5x5
    patches; the live state is 45x45x3 = [75=(a,b,co), 9x9 blocks] per
    sample.  Samples whose max|At(x)| <= lam stay exactly 0 and skip the
    device entirely.
  * conv(y) on the live grid is a 5x5 conv over the 9x9 block grid with
    75 input channels -> 3 outputs = 25 shift-matmuls (K=75, M=3, N=81)
    accumulated in PSUM across 4 tile_position column groups.
  * Momentum commutes with the conv: A(y_tmp_{n+1}) = s_n A(y_new_n)
    - b_n A(y_new_{n-1}), so the conv runs on y_new right after
    thresholding and the scaled images live in two ping-pong SBUF tiles
    (zmov) whose 4 group-slots sit at the 32-aligned partition starts the
    engines require.  The prev tile is rescaled in place by
    r_n = -beta_n/s_{n-1} each iteration.
  * The At-phase w = y_tmp + Wr(A(y_tmp) + bx) - b_ct is THREE accumulating
    matmuls with CONSTANT stationaries: [I75|Wr(bx)|-b_ct] @ ytmp-tile,
    Wr-pattern @ zmov_cur, Wr-pattern @ zmov_prev.
  * y_tmp_{n+1} = s_n y_new_n - b_n y_new_{n-1} is materialized on GpSimd,
    fully overlapped with the conv matmuls.

All 200 iterations run inside ONE tc.For_i hardware loop (2 iterations
per trip, ping-pong buffers); per-iteration scalars (s_n = 1+beta_n,
b_n = beta_n, r_n = -beta_n/s_{n-1}) come from a small SBUF table via
dynamically sliced [P,1] scalar APs.  Program is ~110 instructions vs
~7200 fully unrolled, which slashes both NEFF build/load overhead and
device time.

Each active sample runs on its own NeuronCore (8 cores; extra actives are
handled in additional device rounds).
"""

import math
import os
import sys

import numpy as np

for _p in ("/opt/trn_rl_repo", "/root/.axon_site/_ro/trn_rl_repo"):
    if os.path.isdir(_p) and _p not in sys.path:
        sys.path.insert(0, _p)

N_ITERS = 200
N_CORES = 8
HW = 72
LOW = 9
C = 3
F = 75          # (a,b,co): 5*5*3 live phase-space channels
NP2 = 13        # padded block grid (9 + 2 on each side)
NB = 9          # block grid
NPOS = NB * NB  # 81
SB = 100        # sched table block stride (max 100 trips per parity)


def _betas(n_iters):
    """beta_n = (t_n - 1)/t_{n+1}, bit-exact fp32 mirror of the reference."""
    one, two, four = np.float32(1.0), np.float32(2.0), np.float32(4.0)
    t = np.float32(1.0)
    out = []
    for _ in range(n_iters):
        t_n = (one + np.sqrt(one + four * t * t)) / two
        out.append(np.float32((t - one) / t_n))
        t = t_n
    return out


_DEV_CACHE = {}


def _build_device(n_iters, sb=SB):
    """Build + compile the per-core FISTA program (SPMD, same code all cores).

    sb: sched-table block stride (columns per parity block); the default fits
    N_ITERS=200.  Larger values are only used by timing harnesses that build
    longer-running variants of the same loop."""
    if (n_iters, sb) in _DEV_CACHE:
        return _DEV_CACHE[(n_iters, sb)]
    assert n_iters <= 2 * sb

    import concourse.bacc as bacc
    import concourse.mybir as mybir
    from concourse.bass import ds
    from concourse.tile import TileContext

    f32 = mybir.dt.float32
    Alu = mybir.AluOpType
    ActFn = mybir.ActivationFunctionType

    # shift s -> column group; g0 gets 7 shifts, g1/g2/g3 6.
    grp_of = [s % 4 for s in range(25)]
    by_g = [[s for s in range(25) if grp_of[s] == g] for g in range(4)]
    order = []  # round-robin issue order for concurrency
    for rr in range(7):
        for g in range(4):
            if rr < len(by_g[g]):
                order.append(by_g[g][rr])

    nc = bacc.Bacc(trn_type="TRN2")
    wc_d = nc.dram_tensor("wc", [F, 75], f32, kind="ExternalInput")
    wta_d = nc.dram_tensor("wta", [100, F], f32, kind="ExternalInput")
    wtb_d = nc.dram_tensor("wtb", [99, F], f32, kind="ExternalInput")
    bxo_d = nc.dram_tensor("bxo", [4, NPOS], f32, kind="ExternalInput")
    lam_d = nc.dram_tensor("lam2", [F, 2], f32, kind="ExternalInput")
    sched_d = nc.dram_tensor("sched", [128, 6 * sb], f32, kind="ExternalInput")
    y_d = nc.dram_tensor("y", [F, NPOS], f32, kind="ExternalOutput")

    with TileContext(nc) as tc:
        with tc.tile_pool(name="const", bufs=1) as cpool, \
             tc.tile_pool(name="state", bufs=1) as spool, \
             tc.tile_pool(name="work", bufs=3) as wpool, \
             tc.tile_pool(name="psum", bufs=2, space="PSUM") as ppool:
            wc = cpool.tile([F, 75], f32, tag="wc")
            nc.sync.dma_start(wc[:], wc_d[:])
            wta = cpool.tile([100, F], f32, tag="wta")
            nc.sync.dma_start(wta[:], wta_d[:])
            wtb = cpool.tile([99, F], f32, tag="wtb")
            nc.sync.dma_start(wtb[:], wtb_d[:])
            lam2 = cpool.tile([F, 2], f32, tag="lam")
            nc.sync.dma_start(lam2[:], lam_d[:])
            # one table tile per (scalar, parity): plain ds(i) everywhere —
            # register-addend dynamic offsets cost ~4.5us per op on DVE.
            tabs = [cpool.tile([128, sb], f32, tag=f"tab{b}", name=f"tab{b}")
                    for b in range(6)]
            for b in range(6):
                nc.sync.dma_start(tabs[b][:], sched_d[:, b * sb:(b + 1) * sb])

            # ytmp rows 0:75 = y_tmp; 96:99 = bx (static); 99 = ones.
            ytmp = spool.tile([100, NPOS], f32, tag="ytmp")
            nc.vector.memset(ytmp[:], 0.0)
            nc.sync.dma_start(ytmp[96:100, :], bxo_d[:])
            # zmov ping-pong tiles: rows 32g..32g+2 = conv image of group g
            zmov = [spool.tile([99, NPOS], f32, tag=f"zm{i}", name=f"zm{i}")
                    for i in range(2)]
            # fb: padded scaled state f_n = s_n * y_new_n (ping-pong)
            fb = [spool.tile([F, NP2 * NP2], f32, tag=f"fb{i}", name=f"fb{i}")
                  for i in range(2)]
            for t in zmov + fb:
                nc.vector.memset(t[:], 0.0)

            fbv = [t[:].rearrange("p (r c) -> p r c", c=NP2) for t in fb]
            fb_int = [v[:, 2:2 + NB, 2:2 + NB] for v in fbv]

            def emit_iter(idx, p, dyn):
                """One FISTA iteration. idx: trip index (ScalarValue if dyn
                else python int); p: parity (0 even, 1 odd)."""
                def scol(rows, block):
                    if dyn:
                        return tabs[block][rows, ds(idx, 1)]
                    return tabs[block][rows, idx:idx + 1]

                # blocks: 0 s_e, 1 s_o, 2 g_e, 3 g_o, 4 r_e, 5 r_o
                blk_s, blk_g, blk_r = p, 2 + p, 4 + p
                cur, prev = zmov[1 - p], zmov[p]

                # At-phase: w_n = y_tmp + Wr(A(y_tmp) + bx) - b_ct
                pw = ppool.tile([F, NPOS], f32, tag="pw")
                nc.tensor.matmul(pw[:], wta[:], ytmp[:], start=True, stop=False)
                nc.tensor.matmul(pw[:], wtb[:], cur[:], start=False, stop=False)
                nc.tensor.matmul(pw[:], wtb[:], prev[:], start=False, stop=True)

                # rescale what was "cur" for use as "prev" next iteration:
                # content A(f_{n-1}) -> -g_n A(f_{n-1})
                nc.vector.tensor_scalar_mul(cur[:], cur[:],
                                            scol(slice(0, 99), blk_r))

                # soft threshold: y_new = pw - clamp(pw, +-lam), then the
                # scaled state f_n = s_n * y_new  (all DVE, same engine)
                cl = wpool.tile([F, NPOS], f32, tag="cl")
                nc.vector.tensor_scalar(
                    cl[:], pw[:], lam2[:, 0:1], lam2[:, 1:2], Alu.min, Alu.max)
                yv = wpool.tile([F, NPOS], f32, tag="yv")
                nc.vector.tensor_sub(yv[:], pw[:], cl[:])
                nc.vector.tensor_scalar_mul(fb_int[p], yv[:],
                                            scol(slice(0, 75), blk_s))

                # conv on f_n: 25 shift-matmuls in 4 PSUM column groups
                pz = ppool.tile([128, NPOS], f32, tag="pz")
                seen = [0, 0, 0, 0]
                for s in order:
                    g = grp_of[s]
                    m, nn_ = divmod(s, 5)
                    nc.tensor.matmul(
                        pz[32 * g:32 * g + 3, :],
                        wc[:, 3 * s:3 * s + 3],
                        fbv[p][:, m:m + NB, nn_:nn_ + NB],
                        start=(seen[g] == 0),
                        stop=(seen[g] == len(by_g[g]) - 1),
                        tile_position=(0, 32 * g),
                    )
                    seen[g] += 1

                # plain image copies into the new "cur" (= prev buffer), DVE
                for g in range(4):
                    nc.vector.tensor_copy(prev[32 * g:32 * g + 3, :],
                                          pz[32 * g:32 * g + 3, :])

                # momentum (overlaps conv): y_tmp' = f_n - g_n f_{n-1}
                u = wpool.tile([F, NPOS], f32, tag="u")
                nc.vector.tensor_scalar_mul(
                    u[:], fb_int[1 - p], scol(slice(0, 75), blk_g))
                nc.vector.tensor_sub(ytmp[0:F, :], fb_int[p], u[:])

            trips = n_iters // 2
            tail = n_iters % 2
            if trips > 0:
                with tc.For_i(0, trips, 1) as i:
                    emit_iter(i, 0, True)
                    emit_iter(i, 1, True)
            if tail:
                emit_iter(trips, 0, False)

            last = (n_iters - 1) % 2 if n_iters > 0 else 1
            nc.sync.dma_start(y_d[:], fb_int[last])

    nc.compile()
    _DEV_CACHE[(n_iters, sb)] = nc
    return nc


def _host_tables(w_conv, b_conv, w_ct, b_ct, n_iters=N_ITERS):
    """Device weight tables (sample-independent)."""
    w_rev = w_ct[::-1, ::-1]                      # [a,b,ci,co]
    aa, bb_, cc = np.meshgrid(np.arange(5), np.arange(5), np.arange(C),
                              indexing='ij')
    Wc_all = np.zeros((F, 75), np.float32)
    for s in range(25):
        m, n = divmod(s, 5)
        blk = w_conv[8 * m + aa, 8 * n + bb_, cc, :]      # (5,5,3,3)
        Wc_all[:, 3 * s:3 * s + 3] = blk.reshape(F, C)
    Wr = np.transpose(w_rev, (2, 0, 1, 3)).reshape(C, F)  # [ci, (a,b,co)]

    Wta = np.zeros((100, F), np.float32)
    Wta[0:F, :] = np.eye(F, dtype=np.float32)
    Wta[96:99, :] = Wr                  # bx rows
    Wta[99, :] = np.broadcast_to(-b_ct, (5, 5, C)).reshape(F)
    Wtb = np.zeros((99, F), np.float32)
    for g in range(4):
        Wtb[32 * g:32 * g + 3, :] = Wr

    betas = _betas(max(n_iters, 1))
    s_arr = np.array([np.float32(1.0) + b for b in betas], np.float32)
    b_arr = np.array(betas, np.float32)
    r_arr = np.zeros(len(betas), np.float32)
    for n in range(len(betas)):
        s_prev = s_arr[n - 1] if n > 0 else np.float32(1.0)
        r_arr[n] = np.float32(-(b_arr[n] / s_prev))

    g_arr = -r_arr                                 # g_n = beta_n / s_{n-1}
    sched = np.zeros((128, 6 * SB), np.float32)
    for n in range(n_iters):
        p, i = n % 2, n // 2
        sched[:, p * SB + i] = s_arr[n]            # blocks 0/1: s
        sched[:, (2 + p) * SB + i] = g_arr[n]      # blocks 2/3: g
        sched[:, (4 + p) * SB + i] = r_arr[n]      # blocks 4/5: r
    return Wc_all, Wta, Wtb, w_rev, sched, s_arr


def kernel(x, lam, w_conv, b_conv, w_ct, b_ct):
    from concourse import bass_utils

    x = np.asarray(x, np.float32)
    lam = np.asarray(lam, np.float32)
    w_conv = np.asarray(w_conv, np.float32)
    b_conv = np.asarray(b_conv, np.float32)
    w_ct = np.asarray(w_ct, np.float32)
    b_ct = np.asarray(b_ct, np.float32)
    B = x.shape[0]

    Wc_all, Wta, Wtb, w_rev, sched, s_arr = _host_tables(w_conv, b_conv, w_ct, b_ct)

    # ---- host analysis (exact): c = At(x - b_conv) + b_ct on the live grid
    xb = x - b_conv                               # (B,9,9,3)
    c = np.einsum('abeo,sije->sabo' 'ij', w_rev, xb, optimize=True)
    c = c + b_ct[None, None, None, :, None, None]
    cmax = np.abs(c).max(axis=(1, 2, 3, 4, 5))
    active = cmax > lam * np.float32(1.0 - 1e-5)
    act_idx = np.where(active)[0]

    out = np.zeros((B, HW, HW, C), np.float32)

    # Non-patch positions evolve autonomously: w = y - b_ct per channel.
    # Exact when b_ct == 0 (it is, per the model); otherwise computed here.
    if np.any(b_ct != 0.0):
        betas = _betas(N_ITERS)
        yv = np.zeros((B, C), np.float32)
        yl = np.zeros((B, C), np.float32)
        for n in range(N_ITERS):
            w_np = yv - b_ct[None, :]
            y_new = (np.maximum(w_np - lam[:, None], 0)
                     - np.maximum(-w_np - lam[:, None], 0)).astype(np.float32)
            yv = y_new + np.float32(betas[n]) * (y_new - yl)
            yl = y_new
        mask = np.ones((HW, HW), bool)
        rows = (np.arange(HW) % 8) < 5
        mask[np.ix_(rows, rows)] = False          # live-grid positions
        out[:, mask, :] = yl[:, None, :]

    nc = _build_device(N_ITERS)

    n_rounds = max(1, math.ceil(len(act_idx) / N_CORES))
    zero_bxo = np.zeros((4, NPOS), np.float32)
    zero_bxo[3, :] = 1.0
    one_lam = np.stack([np.ones(F, np.float32), -np.ones(F, np.float32)],
                       axis=1)
    for r in range(n_rounds):
        batch = act_idx[r * N_CORES:(r + 1) * N_CORES]
        in_maps = []
        for k in range(N_CORES):
            if k < len(batch):
                s = int(batch[k])
                bxo = np.empty((4, NPOS), np.float32)
                bxo[0:3] = b_conv[:, None] - x[s].reshape(NPOS, C).T
                bxo[3, :] = 1.0
                lam2 = np.stack([np.full(F, lam[s], np.float32),
                                 np.full(F, -lam[s], np.float32)], axis=1)
            else:
                bxo, lam2 = zero_bxo, one_lam
            in_maps.append({
                "wc": Wc_all, "wta": Wta, "wtb": Wtb,
                "bxo": np.ascontiguousarray(bxo),
                "lam2": np.ascontiguousarray(lam2), "sched": sched,
            })
        res = bass_utils.run_bass_kernel_spmd(nc, in_maps,
                                              core_ids=list(range(N_CORES)))
        for k in range(len(batch)):
            s = int(batch[k])
            ya = (res.results[k]["y"] / s_arr[N_ITERS - 1]
                  ).astype(np.float32).reshape(5, 5, C, NB, NB)
            # out[s, 8I+a, 8J+b, co] = ya[a,b,co,I,J]
            blk = np.transpose(ya, (3, 0, 4, 1, 2))   # (I,a,J,b,co)
            ov = out[s].reshape(NB, 8, NB, 8, C)
            ov[:, :5, :, :5, :] = blk
    return out



# revision 2
# speedup vs baseline: 2.3364x; 2.3364x over previous
"""Trainium2 Bass kernel for nn_CompressedSensingConvolutional (gform-bf16).

Problem: 200 FISTA iterations of
    re    = conv_transpose(x - conv(y_tmp, w_conv, stride 8, SAME) - b_conv,
                           w_ct, stride 8, SAME) + b_ct
    w     = y_tmp - re
    y_new = soft_threshold(w, lam)        (per-sample lam)
    y_tmp = y_new + beta_n (y_new - y_last)
with x: (64,9,9,3), output y_new: (64,72,72,3).

Structure exploited:
  * conv_transpose with 5x5 kernel / stride 8 writes NON-overlapping 5x5
    patches; the live state is 45x45x3 = [75=(a,b,co), 9x9 blocks] per
    sample.  Samples whose max|At(x)| <= lam stay exactly 0 and skip the
    device entirely.
  * w = y_tmp + Wr(conv(y_tmp)) - Wr x_hat - b_ct: the identity + constant
    part is ONE exact fp32 matmul [I; Wr; -b_ct] @ [y_tmp; b_conv-x; 1]
    (the moving tile carries the constants as extra rows); the conv part
    is a 5x5 block-conv with 75->75 kernels G_s = Wc_s @ Wr (rank 3),
    done as 25 accumulating bf16 matmuls over a bf16 shadow of y_tmp.
    bf16 is safe there: the G-term is O(1e-4) of y, so its rounding
    perturbs y by ~1e-5 relative per iteration (measured end-to-end
    rel err vs fp32 < 1e-2 over 200 iters on every active sample).
  * Per iteration 5 DVE ops: clamp, y_new = w - clamp, d = y_new - y_last,
    fused y_tmp = beta*d + y_new (scalar_tensor_tensor), and the bf16
    shadow downcast copy.
  * All 200 iterations run inside ONE tc.For_i hardware loop (UNROLL
    iterations per trip, ping-pong yb buffers); per-iteration beta comes
    from per-slot SBUF tables via plain ds(i) scalar APs.

Each active sample runs on its own NeuronCore (8 cores; extra actives are
handled in additional device rounds).
"""

import math
import os
import sys

import numpy as np

for _p in ("/opt/trn_rl_repo", "/root/.axon_site/_ro/trn_rl_repo"):
    if os.path.isdir(_p) and _p not in sys.path:
        sys.path.insert(0, _p)

N_ITERS = 200
N_CORES = 8
HW = 72
LOW = 9
C = 3
F = 75          # (a,b,co): 5*5*3 live phase-space channels
EXT = 80        # F + 3 bx rows + ones row + 1 spare
NP2 = 13        # padded block grid (9 + 2 on each side)
NB = 9          # block grid
NPOS = NB * NB  # 81
SBG = 128       # beta table block stride (max trips per slot)
UNROLL = 4      # iterations per For_i trip
STAGGERED = True


def _betas(n_iters):
    """beta_n = (t_n - 1)/t_{n+1}, bit-exact fp32 mirror of the reference."""
    one, two, four = np.float32(1.0), np.float32(2.0), np.float32(4.0)
    t = np.float32(1.0)
    out = []
    for _ in range(n_iters):
        t_n = (one + np.sqrt(one + four * t * t)) / two
        out.append(np.float32((t - one) / t_n))
        t = t_n
    return out


_DEV_CACHE = {}


def _build_device(n_iters, unroll=UNROLL, staggered=STAGGERED):
    """Build + compile the per-core FISTA program (SPMD, same code all cores)."""
    key = (n_iters, unroll, staggered)
    if key in _DEV_CACHE:
        return _DEV_CACHE[key]

    import concourse.bacc as bacc
    import concourse.mybir as mybir
    from concourse.bass import ds
    from concourse.tile import TileContext

    f32 = mybir.dt.float32
    b16 = mybir.dt.bfloat16
    Alu = mybir.AluOpType
    sb = SBG
    trips = n_iters // unroll
    rem = n_iters - trips * unroll
    assert trips + (1 if rem else 0) <= sb
    assert unroll % 2 == 0, "yb ping-pong needs even unroll"

    nc = bacc.Bacc(trn_type="TRN2")
    wa_d = nc.dram_tensor("wa", [EXT, F], f32, kind="ExternalInput")
    wgb_d = nc.dram_tensor("wgb", [F, 25 * F], b16, kind="ExternalInput")
    bxp_d = nc.dram_tensor("bxp", [4, NPOS], f32, kind="ExternalInput")
    lam_d = nc.dram_tensor("lam2", [F, 2], f32, kind="ExternalInput")
    sched_d = nc.dram_tensor("sched", [128, 8 * sb], f32, kind="ExternalInput")
    y_d = nc.dram_tensor("y", [F, NPOS], f32, kind="ExternalOutput")

    with TileContext(nc) as tc:
        with tc.tile_pool(name="const", bufs=1) as cpool, \
             tc.tile_pool(name="state", bufs=1) as spool, \
             tc.tile_pool(name="work", bufs=3) as wpool, \
             tc.tile_pool(name="psum", bufs=2, space="PSUM") as ppool:
            wa = cpool.tile([EXT, F], f32, tag="wa")
            nc.sync.dma_start(wa[:], wa_d[:])
            wgb = cpool.tile([F, 25 * F], b16, tag="wgb")
            nc.sync.dma_start(wgb[:], wgb_d[:])
            lam2 = cpool.tile([F, 2], f32, tag="lam")
            nc.sync.dma_start(lam2[:], lam_d[:])
            tabs = [cpool.tile([128, sb], f32, tag=f"tab{b}", name=f"tab{b}")
                    for b in range(unroll)]
            for b in range(unroll):
                nc.sync.dma_start(tabs[b][:], sched_d[:, b * sb:(b + 1) * sb])

            # fp32 un-padded extended state [80, 81]: rows 0:75 y_tmp,
            # 75:78 = b_conv - x, row 78 = ones, row 79 = 0.
            ytex = spool.tile([EXT, NPOS], f32, tag="ytex")
            nc.vector.memset(ytex[:], 0.0)
            nc.sync.dma_start(ytex[F:F + 4, :], bxp_d[0:4])
            # bf16 padded shadow of y_tmp for the G matmuls
            ypb = spool.tile([F, NP2 * NP2], b16, tag="ypb")
            nc.vector.memset(ypb[:], 0.0)
            ypbv = ypb[:].rearrange("p (r c) -> p r c", c=NP2)
            ypb_int = ypbv[:, 2:2 + NB, 2:2 + NB]
            yb = [spool.tile([F, NPOS], f32, tag=f"yb{i}", name=f"yb{i}")
                  for i in range(2)]
            for t in yb:
                nc.vector.memset(t[:], 0.0)

            def emit_iter(idx, p, tab, dyn):
                if dyn:
                    sc_b = tab[0:F, ds(idx, 1)]
                else:
                    sc_b = tab[0:F, idx:idx + 1]

                # G MMs first (gated only on the bf16 shadow), exact
                # identity+const MM last (gated on the fp32 state).
                pw = ppool.tile([F, NPOS], f32, tag="pw")
                for s in range(25):
                    m, nn_ = divmod(s, 5)
                    nc.tensor.matmul(
                        pw[:], wgb[:, F * s:F * (s + 1)],
                        ypbv[:, m:m + NB, nn_:nn_ + NB],
                        start=(s == 0), stop=False)
                nc.tensor.matmul(pw[:], wa[:], ytex[:], start=False, stop=True)

                cl = wpool.tile([F, NPOS], f32, tag="cl")
                nc.vector.tensor_scalar(
                    cl[:], pw[:], lam2[:, 0:1], lam2[:, 1:2], Alu.min, Alu.max)
                nc.vector.tensor_sub(yb[p][:], pw[:], cl[:])
                d = wpool.tile([F, NPOS], f32, tag="d")
                nc.vector.tensor_sub(d[:], yb[p][:], yb[1 - p][:])
                d_rc = d[:].rearrange("p (r c) -> p r c", c=NB)
                yb_rc = yb[p][:].rearrange("p (r c) -> p r c", c=NB)
                # bf16 shadow y_tmp = y_new + beta*d straight into the pad
                # (unblocks next iteration's G MMs), then the fp32 y_tmp.
                nc.vector.scalar_tensor_tensor(
                    ypb_int[:, :, :], d_rc, sc_b, yb_rc, Alu.mult, Alu.add)
                nc.vector.scalar_tensor_tensor(
                    ytex[0:F, :], d[:], sc_b, yb[p][:], Alu.mult, Alu.add)

            if trips > 0:
                with tc.For_i(0, trips, 1, staggered_reset=staggered) as i:
                    for k in range(unroll):
                        emit_iter(i, k % 2, tabs[k], True)
            for k in range(rem):
                emit_iter(trips, k % 2, tabs[k], False)

            last = (n_iters - 1) % 2 if n_iters > 0 else 1
            nc.sync.dma_start(y_d[:], yb[last][:])

    nc.compile()
    _DEV_CACHE[key] = nc
    return nc


def _host_tables(w_conv, b_conv, w_ct, b_ct, n_iters=N_ITERS, unroll=UNROLL):
    """Device weight tables (sample-independent)."""
    import ml_dtypes
    w_rev = w_ct[::-1, ::-1]                      # [a,b,ci,co]
    Wr = np.transpose(w_rev, (2, 0, 1, 3)).reshape(C, F).astype(np.float32)
    aa, bb_, cc = np.meshgrid(np.arange(5), np.arange(5), np.arange(C),
                              indexing='ij')
    Wgb = np.zeros((F, 25 * F), np.float32)
    for s in range(25):
        m, n = divmod(s, 5)
        Wcs = w_conv[8 * m + aa, 8 * n + bb_, cc, :].reshape(F, C)
        Wgb[:, F * s:F * (s + 1)] = (
            Wcs.astype(np.float64) @ Wr.astype(np.float64)).astype(np.float32)
    Wgb = Wgb.astype(ml_dtypes.bfloat16)
    Wa = np.zeros((EXT, F), np.float32)
    Wa[0:F] = np.eye(F, dtype=np.float32)
    Wa[F:F + 3] = Wr                               # bx rows: Wr(b_conv - x)
    Wa[F + 3] = np.broadcast_to(-b_ct, (5, 5, C)).reshape(F)

    betas = _betas(max(n_iters, 1))
    sched = np.zeros((128, 8 * SBG), np.float32)
    for n in range(n_iters):
        k, i = n % unroll, n // unroll
        sched[:, k * SBG + i] = betas[n]
    return Wa, Wgb, w_rev, sched


def kernel(x, lam, w_conv, b_conv, w_ct, b_ct):
    from concourse import bass_utils

    x = np.asarray(x, np.float32)
    lam = np.asarray(lam, np.float32)
    w_conv = np.asarray(w_conv, np.float32)
    b_conv = np.asarray(b_conv, np.float32)
    w_ct = np.asarray(w_ct, np.float32)
    b_ct = np.asarray(b_ct, np.float32)
    B = x.shape[0]

    Wa, Wgb, w_rev, sched = _host_tables(w_conv, b_conv, w_ct, b_ct)

    # ---- host analysis (exact): c = At(x - b_conv) + b_ct on the live grid
    xb = x - b_conv                               # (B,9,9,3)
    c = np.einsum('abeo,sije->sabo' 'ij', w_rev, xb, optimize=True)
    c = c + b_ct[None, None, None, :, None, None]
    cmax = np.abs(c).max(axis=(1, 2, 3, 4, 5))
    active = cmax > lam * np.float32(1.0 - 1e-5)
    act_idx = np.where(active)[0]

    out = np.zeros((B, HW, HW, C), np.float32)

    # Non-patch positions evolve autonomously: w = y - b_ct per channel.
    # Exact when b_ct == 0 (it is, per the model); otherwise computed here.
    if np.any(b_ct != 0.0):
        betas = _betas(N_ITERS)
        yv = np.zeros((B, C), np.float32)
        yl = np.zeros((B, C), np.float32)
        for n in range(N_ITERS):
            w_np = yv - b_ct[None, :]
            y_new = (np.maximum(w_np - lam[:, None], 0)
                     - np.maximum(-w_np - lam[:, None], 0)).astype(np.float32)
            yv = y_new + np.float32(betas[n]) * (y_new - yl)
            yl = y_new
        mask = np.ones((HW, HW), bool)
        rows = (np.arange(HW) % 8) < 5
        mask[np.ix_(rows, rows)] = False          # live-grid positions
        out[:, mask, :] = yl[:, None, :]

    nc = _build_device(N_ITERS)

    n_rounds = max(1, math.ceil(len(act_idx) / N_CORES))
    zero_bxp = np.zeros((4, NPOS), np.float32)
    zero_bxp[3, :] = 1.0
    one_lam = np.stack([np.ones(F, np.float32), -np.ones(F, np.float32)],
                       axis=1)
    for r in range(n_rounds):
        batch = act_idx[r * N_CORES:(r + 1) * N_CORES]
        in_maps = []
        for k in range(N_CORES):
            if k < len(batch):
                s = int(batch[k])
                bxp = np.zeros((4, NPOS), np.float32)
                bxp[0:3] = b_conv[:, None] - x[s].reshape(NPOS, C).T
                bxp[3, :] = 1.0
                lam2 = np.stack([np.full(F, lam[s], np.float32),
                                 np.full(F, -lam[s], np.float32)], axis=1)
            else:
                bxp, lam2 = zero_bxp, one_lam
            in_maps.append({
                "wa": Wa, "wgb": Wgb, "bxp": np.ascontiguousarray(bxp),
                "lam2": np.ascontiguousarray(lam2), "sched": sched,
            })
        res = bass_utils.run_bass_kernel_spmd(nc, in_maps,
                                              core_ids=list(range(N_CORES)))
        for k in range(len(batch)):
            s = int(batch[k])
            ya = np.asarray(res.results[k]["y"], np.float32
                            ).reshape(5, 5, C, NB, NB)
            # out[s, 8I+a, 8J+b, co] = ya[a,b,co,I,J]
            blk = np.transpose(ya, (3, 0, 4, 1, 2))   # (I,a,J,b,co)
            ov = out[s].reshape(NB, 8, NB, 8, C)
            ov[:, :5, :, :5, :] = blk
    return out


# revision 4
# speedup vs baseline: 6.0270x; 2.5796x over previous
"""Trainium2 Bass kernel for nn_CompressedSensingConvolutional (gform-bf16).

Problem: 200 FISTA iterations of
    re    = conv_transpose(x - conv(y_tmp, w_conv, stride 8, SAME) - b_conv,
                           w_ct, stride 8, SAME) + b_ct
    w     = y_tmp - re
    y_new = soft_threshold(w, lam)        (per-sample lam)
    y_tmp = y_new + beta_n (y_new - y_last)
with x: (64,9,9,3), output y_new: (64,72,72,3).

Structure exploited:
  * conv_transpose with 5x5 kernel / stride 8 writes NON-overlapping 5x5
    patches; the live state is 45x45x3 = [75=(a,b,co), 9x9 blocks] per
    sample.  Samples whose max|At(x)| <= lam stay exactly 0 and skip the
    device entirely.
  * w = y_tmp + Wr(conv(y_tmp)) - Wr x_hat - b_ct: the identity + constant
    part is ONE exact fp32 matmul [I; Wr; -b_ct] @ [y_tmp; b_conv-x; 1]
    (the moving tile carries the constants as extra rows); the conv part
    is a 5x5 block-conv with 75->75 kernels G_s = Wc_s @ Wr (rank 3),
    done as 25 accumulating bf16 matmuls over a bf16 shadow of y_tmp.
    bf16 is safe there: the G-term is O(1e-4) of y, so its rounding
    perturbs y by ~1e-5 relative per iteration (measured end-to-end
    rel err vs fp32 < 1e-2 over 200 iters on every active sample).
  * Momentum is re-associated as y_tmp = (1+beta_n) y_new - q_n with
    q_n = beta_n y_last precomputed on DVE while the PE block runs, so the
    post-matmul DVE depth is only clamp -> y_new -> fused
    (s*y_new - q) writes (bf16 shadow + fp32 state); 5 DVE ops total.
  * All 200 iterations run inside ONE tc.For_i hardware loop (UNROLL
    iterations per trip, ping-pong yb buffers); per-iteration beta comes
    from per-slot SBUF tables via plain ds(i) scalar APs.

Each active sample runs on its own NeuronCore (8 cores; extra actives are
handled in additional device rounds).
"""

import math
import os
import sys

import numpy as np

for _p in ("/opt/trn_rl_repo", "/root/.axon_site/_ro/trn_rl_repo"):
    if os.path.isdir(_p) and _p not in sys.path:
        sys.path.insert(0, _p)

N_ITERS = 200
N_CORES = 8
HW = 72
LOW = 9
C = 3
F = 75          # (a,b,co): 5*5*3 live phase-space channels
EXT = 80        # F + 3 bx rows + ones row + 1 spare
NP2 = 13        # padded block grid (9 + 2 on each side)
NB = 9          # block grid
NPOS = NB * NB  # 81
SBG = 128       # beta table block stride (max trips per slot)
UNROLL = 2      # iterations per For_i trip
STAGGERED = True


def _betas(n_iters):
    """beta_n = (t_n - 1)/t_{n+1}, bit-exact fp32 mirror of the reference."""
    one, two, four = np.float32(1.0), np.float32(2.0), np.float32(4.0)
    t = np.float32(1.0)
    out = []
    for _ in range(n_iters):
        t_n = (one + np.sqrt(one + four * t * t)) / two
        out.append(np.float32((t - one) / t_n))
        t = t_n
    return out


_DEV_CACHE = {}


def _build_device(n_iters, unroll=UNROLL, staggered=STAGGERED, sb=None):
    """Build + compile the per-core FISTA program (SPMD, same code all cores).

    sb: beta-table block stride (columns per unroll slot); default SBG fits
    N_ITERS=200.  Larger values are only used by timing harnesses that build
    longer-running variants of the same loop."""
    key = (n_iters, unroll, staggered, sb)
    if key in _DEV_CACHE:
        return _DEV_CACHE[key]

    import concourse.bacc as bacc
    import concourse.mybir as mybir
    from concourse.bass import ds
    from concourse.tile import TileContext

    f32 = mybir.dt.float32
    b16 = mybir.dt.bfloat16
    Alu = mybir.AluOpType
    if sb is None:
        sb = SBG
    trips = n_iters // unroll
    rem = n_iters - trips * unroll
    assert trips + (1 if rem else 0) <= sb
    assert unroll % 2 == 0, "yb ping-pong needs even unroll"
    assert rem == 0 or trips == 0, "q-table layout needs n_iters % unroll == 0"
    assert 2 * unroll <= 8, "sched has 8 slots: s + next-beta per position"

    nc = bacc.Bacc(trn_type="TRN2")
    wa_d = nc.dram_tensor("wa", [EXT, F], f32, kind="ExternalInput")
    wgb_d = nc.dram_tensor("wgb", [F, 25 * F], b16, kind="ExternalInput")
    bxp_d = nc.dram_tensor("bxp", [4, NPOS], f32, kind="ExternalInput")
    lam_d = nc.dram_tensor("lam2", [F, 2], f32, kind="ExternalInput")
    sched_d = nc.dram_tensor("sched", [128, 8 * sb], f32, kind="ExternalInput")
    y_d = nc.dram_tensor("y", [F, NPOS], f32, kind="ExternalOutput")

    with TileContext(nc) as tc:
        with tc.tile_pool(name="const", bufs=1) as cpool, \
             tc.tile_pool(name="state", bufs=1) as spool, \
             tc.tile_pool(name="work", bufs=3) as wpool, \
             tc.tile_pool(name="psum", bufs=2, space="PSUM") as ppool:
            wa = cpool.tile([EXT, F], f32, tag="wa")
            nc.sync.dma_start(wa[:], wa_d[:])
            wgb = cpool.tile([F, 25 * F], b16, tag="wgb")
            nc.sync.dma_start(wgb[:], wgb_d[:])
            lam2 = cpool.tile([F, 2], f32, tag="lam")
            nc.sync.dma_start(lam2[:], lam_d[:])
            stab = [cpool.tile([128, sb], f32, tag=f"st{b}", name=f"st{b}")
                    for b in range(unroll)]
            qtab = [cpool.tile([128, sb], f32, tag=f"qt{b}", name=f"qt{b}")
                    for b in range(unroll)]
            for b in range(unroll):
                nc.sync.dma_start(stab[b][:], sched_d[:, b * sb:(b + 1) * sb])
                nc.sync.dma_start(
                    qtab[b][:],
                    sched_d[:, (unroll + b) * sb:(unroll + b + 1) * sb])

            # fp32 un-padded extended state [80, 81]: rows 0:75 y_tmp,
            # 75:78 = b_conv - x, row 78 = ones, row 79 = 0.
            ytex = spool.tile([EXT, NPOS], f32, tag="ytex")
            nc.vector.memset(ytex[:], 0.0)
            nc.sync.dma_start(ytex[F:F + 4, :], bxp_d[0:4])
            # bf16 padded shadow of y_tmp for the G matmuls
            ypb = spool.tile([F, NP2 * NP2], b16, tag="ypb")
            nc.vector.memset(ypb[:], 0.0)
            ypbv = ypb[:].rearrange("p (r c) -> p r c", c=NP2)
            ypb_int = ypbv[:, 2:2 + NB, 2:2 + NB]
            yb = [spool.tile([F, NPOS], f32, tag=f"yb{i}", name=f"yb{i}")
                  for i in range(2)]
            for t in yb:
                nc.vector.memset(t[:], 0.0)
            ybv = [t[:].rearrange("p (r c) -> p r c", c=NB) for t in yb]
            # q ping-pong: position k consumes qt2[k%2], writes qt2[(k+1)%2]
            qt2 = [spool.tile([F, NPOS], f32, tag=f"q{i}", name=f"q{i}")
                   for i in range(2)]
            for t in qt2:
                nc.vector.memset(t[:], 0.0)
            qv = [t[:].rearrange("p (r c) -> p r c", c=NB) for t in qt2]

            def emit_iter(idx, k, dyn):
                p = k % 2
                if dyn:
                    sc_s = stab[k][0:F, ds(idx, 1)]
                    sc_q = qtab[k][0:F, ds(idx, 1)]
                else:
                    sc_s = stab[k][0:F, idx:idx + 1]
                    sc_q = qtab[k][0:F, idx:idx + 1]

                # G MMs first (gated only on the bf16 shadow), exact
                # identity+const MM last (gated on the fp32 state).
                pw = ppool.tile([F, NPOS], f32, tag="pw")
                for s in range(25):
                    m, nn_ = divmod(s, 5)
                    nc.tensor.matmul(
                        pw[:], wgb[:, F * s:F * (s + 1)],
                        ypbv[:, m:m + NB, nn_:nn_ + NB],
                        start=(s == 0), stop=False)
                nc.tensor.matmul(pw[:], wa[:], ytex[:], start=False, stop=True)

                cl = wpool.tile([F, NPOS], f32, tag="cl")
                nc.vector.tensor_scalar(
                    cl[:], pw[:], lam2[:, 0:1], lam2[:, 1:2], Alu.min, Alu.max)
                nc.vector.tensor_sub(yb[p][:], pw[:], cl[:])
                # momentum re-associated: y_tmp = s*y_new - q with
                # q = beta_next*y_new precomputed under the next PE block.
                # bf16 shadow first (unblocks next iteration's G MMs), then
                # the fp32 y_tmp for MM1, then q for the next iteration.
                nc.vector.scalar_tensor_tensor(
                    ypb_int[:, :, :], ybv[p], sc_s, qv[k % 2],
                    Alu.mult, Alu.subtract)
                nc.vector.scalar_tensor_tensor(
                    ytex[0:F, :], yb[p][:], sc_s, qt2[k % 2][:],
                    Alu.mult, Alu.subtract)
                nc.vector.tensor_scalar_mul(qt2[(k + 1) % 2][:], yb[p][:],
                                            sc_q)

            if trips > 0:
                with tc.For_i(0, trips, 1, staggered_reset=staggered) as i:
                    for k in range(unroll):
                        emit_iter(i, k, True)
            for k in range(rem):
                emit_iter(trips, k, False)

            last = (n_iters - 1) % 2 if n_iters > 0 else 1
            nc.sync.dma_start(y_d[:], yb[last][:])

    nc.compile()
    _DEV_CACHE[key] = nc
    return nc


def _host_tables(w_conv, b_conv, w_ct, b_ct, n_iters=N_ITERS, unroll=UNROLL):
    """Device weight tables (sample-independent)."""
    import ml_dtypes
    w_rev = w_ct[::-1, ::-1]                      # [a,b,ci,co]
    Wr = np.transpose(w_rev, (2, 0, 1, 3)).reshape(C, F).astype(np.float32)
    aa, bb_, cc = np.meshgrid(np.arange(5), np.arange(5), np.arange(C),
                              indexing='ij')
    Wgb = np.zeros((F, 25 * F), np.float32)
    for s in range(25):
        m, n = divmod(s, 5)
        Wcs = w_conv[8 * m + aa, 8 * n + bb_, cc, :].reshape(F, C)
        Wgb[:, F * s:F * (s + 1)] = (
            Wcs.astype(np.float64) @ Wr.astype(np.float64)).astype(np.float32)
    Wgb = Wgb.astype(ml_dtypes.bfloat16)
    Wa = np.zeros((EXT, F), np.float32)
    Wa[0:F] = np.eye(F, dtype=np.float32)
    Wa[F:F + 3] = Wr                               # bx rows: Wr(b_conv - x)
    Wa[F + 3] = np.broadcast_to(-b_ct, (5, 5, C)).reshape(F)

    betas = _betas(max(n_iters + 1, 1))
    sched = np.zeros((128, 8 * SBG), np.float32)
    for n in range(n_iters):
        k, i = n % unroll, n // unroll
        sched[:, k * SBG + i] = np.float32(1.0) + betas[n]   # s = 1 + beta
        sched[:, (unroll + k) * SBG + i] = betas[n + 1]      # next beta (q)
    return Wa, Wgb, w_rev, sched


def kernel(x, lam, w_conv, b_conv, w_ct, b_ct):
    from concourse import bass_utils

    x = np.asarray(x, np.float32)
    lam = np.asarray(lam, np.float32)
    w_conv = np.asarray(w_conv, np.float32)
    b_conv = np.asarray(b_conv, np.float32)
    w_ct = np.asarray(w_ct, np.float32)
    b_ct = np.asarray(b_ct, np.float32)
    B = x.shape[0]

    Wa, Wgb, w_rev, sched = _host_tables(w_conv, b_conv, w_ct, b_ct)

    # ---- host analysis (exact): c = At(x - b_conv) + b_ct on the live grid
    xb = x - b_conv                               # (B,9,9,3)
    c = np.einsum('abeo,sije->sabo' 'ij', w_rev, xb, optimize=True)
    c = c + b_ct[None, None, None, :, None, None]
    cmax = np.abs(c).max(axis=(1, 2, 3, 4, 5))
    active = cmax > lam * np.float32(1.0 - 1e-5)
    act_idx = np.where(active)[0]

    out = np.zeros((B, HW, HW, C), np.float32)

    # Non-patch positions evolve autonomously: w = y - b_ct per channel.
    # Exact when b_ct == 0 (it is, per the model); otherwise computed here.
    if np.any(b_ct != 0.0):
        betas = _betas(N_ITERS)
        yv = np.zeros((B, C), np.float32)
        yl = np.zeros((B, C), np.float32)
        for n in range(N_ITERS):
            w_np = yv - b_ct[None, :]
            y_new = (np.maximum(w_np - lam[:, None], 0)
                     - np.maximum(-w_np - lam[:, None], 0)).astype(np.float32)
            yv = y_new + np.float32(betas[n]) * (y_new - yl)
            yl = y_new
        mask = np.ones((HW, HW), bool)
        rows = (np.arange(HW) % 8) < 5
        mask[np.ix_(rows, rows)] = False          # live-grid positions
        out[:, mask, :] = yl[:, None, :]

    nc = _build_device(N_ITERS)

    n_rounds = max(1, math.ceil(len(act_idx) / N_CORES))
    zero_bxp = np.zeros((4, NPOS), np.float32)
    zero_bxp[3, :] = 1.0
    one_lam = np.stack([np.ones(F, np.float32), -np.ones(F, np.float32)],
                       axis=1)
    for r in range(n_rounds):
        batch = act_idx[r * N_CORES:(r + 1) * N_CORES]
        in_maps = []
        for k in range(N_CORES):
            if k < len(batch):
                s = int(batch[k])
                bxp = np.zeros((4, NPOS), np.float32)
                bxp[0:3] = b_conv[:, None] - x[s].reshape(NPOS, C).T
                bxp[3, :] = 1.0
                lam2 = np.stack([np.full(F, lam[s], np.float32),
                                 np.full(F, -lam[s], np.float32)], axis=1)
            else:
                bxp, lam2 = zero_bxp, one_lam
            in_maps.append({
                "wa": Wa, "wgb": Wgb, "bxp": np.ascontiguousarray(bxp),
                "lam2": np.ascontiguousarray(lam2), "sched": sched,
            })
        res = bass_utils.run_bass_kernel_spmd(nc, in_maps,
                                              core_ids=list(range(N_CORES)))
        for k in range(len(batch)):
            s = int(batch[k])
            ya = np.asarray(res.results[k]["y"], np.float32
                            ).reshape(5, 5, C, NB, NB)
            # out[s, 8I+a, 8J+b, co] = ya[a,b,co,I,J]
            blk = np.transpose(ya, (3, 0, 4, 1, 2))   # (I,a,J,b,co)
            ov = out[s].reshape(NB, 8, NB, 8, C)
            ov[:, :5, :, :5, :] = blk
    return out


# revision 5
# speedup vs baseline: 67.8489x; 11.2575x over previous
"""Trainium2 Bass kernel for nn_CompressedSensingConvolutional (gform-bf16).

Problem: 200 FISTA iterations of
    re    = conv_transpose(x - conv(y_tmp, w_conv, stride 8, SAME) - b_conv,
                           w_ct, stride 8, SAME) + b_ct
    w     = y_tmp - re
    y_new = soft_threshold(w, lam)        (per-sample lam)
    y_tmp = y_new + beta_n (y_new - y_last)
with x: (64,9,9,3), output y_new: (64,72,72,3).

Structure exploited:
  * conv_transpose with 5x5 kernel / stride 8 writes NON-overlapping 5x5
    patches; the live state is 45x45x3 = [75=(a,b,co), 9x9 blocks] per
    sample.  Samples whose max|At(x)| <= lam stay exactly 0 and skip the
    device entirely.
  * w = y_tmp + Wr(conv(y_tmp)) - Wr x_hat - b_ct: the identity + constant
    part is ONE exact fp32 matmul [I; Wr; -b_ct] @ [y_tmp; b_conv-x; 1]
    (the moving tile carries the constants as extra rows); the conv part
    is a 5x5 block-conv with 75->75 kernels G_s = Wc_s @ Wr (rank 3),
    done as 25 accumulating bf16 matmuls over a bf16 shadow of y_tmp.
    bf16 is safe there: the G-term is O(1e-4) of y, so its rounding
    perturbs y by ~1e-5 relative per iteration (measured end-to-end
    rel err vs fp32 < 1e-2 over 200 iters on every active sample).
  * Momentum is re-associated as y_tmp = (1+beta_n) y_new - q_n with
    q_n = beta_n y_last precomputed on DVE while the PE block runs, so the
    post-matmul DVE depth is only clamp -> y_new -> fused
    (s*y_new - q) writes (bf16 shadow + fp32 state); 5 DVE ops total.
  * All 200 iterations run inside ONE tc.For_i hardware loop (UNROLL
    iterations per trip, ping-pong yb buffers); per-iteration beta comes
    from per-slot SBUF tables via plain ds(i) scalar APs.

Each active sample runs on its own NeuronCore (8 cores; extra actives are
handled in additional device rounds).
"""

import math
import os
import sys

import numpy as np

for _p in ("/opt/trn_rl_repo", "/root/.axon_site/_ro/trn_rl_repo"):
    if os.path.isdir(_p) and _p not in sys.path:
        sys.path.insert(0, _p)

N_ITERS = 200
N_CORES = 8
HW = 72
LOW = 9
C = 3
F = 75          # (a,b,co): 5*5*3 live phase-space channels
EXT = 80        # F + 3 bx rows + ones row + 1 spare
NP2 = 13        # padded block grid (9 + 2 on each side)
NB = 9          # block grid
NPOS = NB * NB  # 81
SBG = 128       # beta table block stride (max trips per slot)
UNROLL = 2      # iterations per For_i trip
STAGGERED = True


def _betas(n_iters):
    """beta_n = (t_n - 1)/t_{n+1}, bit-exact fp32 mirror of the reference."""
    one, two, four = np.float32(1.0), np.float32(2.0), np.float32(4.0)
    t = np.float32(1.0)
    out = []
    for _ in range(n_iters):
        t_n = (one + np.sqrt(one + four * t * t)) / two
        out.append(np.float32((t - one) / t_n))
        t = t_n
    return out


_DEV_CACHE = {}


def _build_device(n_iters, unroll=UNROLL, staggered=STAGGERED, sb=None):
    """Build + compile the per-core FISTA program (SPMD, same code all cores).

    sb: beta-table block stride (columns per unroll slot); default SBG fits
    N_ITERS=200.  Larger values are only used by timing harnesses that build
    longer-running variants of the same loop."""
    key = (n_iters, unroll, staggered, sb)
    if key in _DEV_CACHE:
        return _DEV_CACHE[key]

    import concourse.bacc as bacc
    import concourse.mybir as mybir
    from concourse.bass import ds
    from concourse.tile import TileContext

    f32 = mybir.dt.float32
    b16 = mybir.dt.bfloat16
    Alu = mybir.AluOpType
    if sb is None:
        sb = SBG
    trips = n_iters // unroll
    rem = n_iters - trips * unroll
    assert trips + (1 if rem else 0) <= sb
    assert unroll % 2 == 0, "yb ping-pong needs even unroll"
    assert rem == 0 or trips == 0, "q-table layout needs n_iters % unroll == 0"
    assert 2 * unroll <= 8, "sched has 8 slots: s + next-beta per position"

    nc = bacc.Bacc(trn_type="TRN2")
    wa_d = nc.dram_tensor("wa", [EXT, F], f32, kind="ExternalInput")
    wgb_d = nc.dram_tensor("wgb", [F, 25 * F], b16, kind="ExternalInput")
    bxp_d = nc.dram_tensor("bxp", [4, NPOS], f32, kind="ExternalInput")
    lam_d = nc.dram_tensor("lam2", [F, 2], f32, kind="ExternalInput")
    sched_d = nc.dram_tensor("sched", [128, 8 * sb], f32, kind="ExternalInput")
    y_d = nc.dram_tensor("y", [F, NPOS], f32, kind="ExternalOutput")

    with TileContext(nc) as tc:
        with tc.tile_pool(name="const", bufs=1) as cpool, \
             tc.tile_pool(name="state", bufs=1) as spool, \
             tc.tile_pool(name="work", bufs=3) as wpool, \
             tc.tile_pool(name="psum", bufs=2, space="PSUM") as ppool:
            wa = cpool.tile([EXT, F], f32, tag="wa")
            nc.sync.dma_start(wa[:], wa_d[:])
            wgb = cpool.tile([F, 25 * F], b16, tag="wgb")
            nc.sync.dma_start(wgb[:], wgb_d[:])
            lam2 = cpool.tile([F, 2], f32, tag="lam")
            nc.sync.dma_start(lam2[:], lam_d[:])
            stab = [cpool.tile([128, sb], f32, tag=f"st{b}", name=f"st{b}")
                    for b in range(unroll)]
            qtab = [cpool.tile([128, sb], f32, tag=f"qt{b}", name=f"qt{b}")
                    for b in range(unroll)]
            for b in range(unroll):
                nc.sync.dma_start(stab[b][:], sched_d[:, b * sb:(b + 1) * sb])
                nc.sync.dma_start(
                    qtab[b][:],
                    sched_d[:, (unroll + b) * sb:(unroll + b + 1) * sb])

            # fp32 un-padded extended state [80, 81]: rows 0:75 y_tmp,
            # 75:78 = b_conv - x, row 78 = ones, row 79 = 0.
            ytex = spool.tile([EXT, NPOS], f32, tag="ytex")
            nc.vector.memset(ytex[:], 0.0)
            nc.sync.dma_start(ytex[F:F + 4, :], bxp_d[0:4])
            # bf16 padded shadow of y_tmp for the G matmuls
            ypb = spool.tile([F, NP2 * NP2], b16, tag="ypb")
            nc.vector.memset(ypb[:], 0.0)
            ypbv = ypb[:].rearrange("p (r c) -> p r c", c=NP2)
            ypb_int = ypbv[:, 2:2 + NB, 2:2 + NB]
            yb = [spool.tile([F, NPOS], f32, tag=f"yb{i}", name=f"yb{i}")
                  for i in range(2)]
            for t in yb:
                nc.vector.memset(t[:], 0.0)
            ybv = [t[:].rearrange("p (r c) -> p r c", c=NB) for t in yb]
            # q ping-pong: position k consumes qt2[k%2], writes qt2[(k+1)%2]
            qt2 = [spool.tile([F, NPOS], f32, tag=f"q{i}", name=f"q{i}")
                   for i in range(2)]
            for t in qt2:
                nc.vector.memset(t[:], 0.0)
            qv = [t[:].rearrange("p (r c) -> p r c", c=NB) for t in qt2]

            def emit_mms(idx, k):
                # G MMs first (gated only on the bf16 shadow), exact
                # identity+const MM last (gated on the fp32 state).
                pw = ppool.tile([F, NPOS], f32, tag="pw")
                for s in range(25):
                    m, nn_ = divmod(s, 5)
                    nc.tensor.matmul(
                        pw[:], wgb[:, F * s:F * (s + 1)],
                        ypbv[:, m:m + NB, nn_:nn_ + NB],
                        start=(s == 0), stop=False)
                nc.tensor.matmul(pw[:], wa[:], ytex[:], start=False, stop=True)
                return pw

            def emit_iter(idx, k, dyn, pw=None):
                p = k % 2
                if dyn:
                    sc_s = stab[k][0:F, ds(idx, 1)]
                    sc_q = qtab[k][0:F, ds(idx, 1)]
                else:
                    sc_s = stab[k][0:F, idx:idx + 1]
                    sc_q = qtab[k][0:F, idx:idx + 1]

                if pw is None:
                    pw = emit_mms(idx, k)

                cl = wpool.tile([F, NPOS], f32, tag="cl")
                nc.vector.tensor_scalar(
                    cl[:], pw[:], lam2[:, 0:1], lam2[:, 1:2], Alu.min, Alu.max)
                nc.vector.tensor_sub(yb[p][:], pw[:], cl[:])
                # momentum re-associated: y_tmp = s*y_new - q with
                # q = beta_next*y_new precomputed under the next PE block.
                # bf16 shadow first (unblocks next iteration's G MMs), then
                # the fp32 y_tmp for MM1, then q for the next iteration.
                nc.vector.scalar_tensor_tensor(
                    ypb_int[:, :, :], ybv[p], sc_s, qv[k % 2],
                    Alu.mult, Alu.subtract)
                nc.vector.scalar_tensor_tensor(
                    ytex[0:F, :], yb[p][:], sc_s, qt2[k % 2][:],
                    Alu.mult, Alu.subtract)
                nc.vector.tensor_scalar_mul(qt2[(k + 1) % 2][:], yb[p][:],
                                            sc_q)

            if trips > 0:
                with tc.For_i(0, trips, 1, staggered_reset=staggered) as i:
                    if staggered and unroll == 2:
                        # stage boundaries aligned to the PE/DVE phases:
                        # [iter0 MMs | iter0 DVE | iter1 MMs | iter1 DVE]
                        pw0 = emit_mms(i, 0)
                        tc.stage_boundary()
                        emit_iter(i, 0, True, pw=pw0)
                        tc.stage_boundary()
                        pw1 = emit_mms(i, 1)
                        tc.stage_boundary()
                        emit_iter(i, 1, True, pw=pw1)
                    else:
                        for k in range(unroll):
                            emit_iter(i, k, True)
            for k in range(rem):
                emit_iter(trips, k, False)

            last = (n_iters - 1) % 2 if n_iters > 0 else 1
            nc.sync.dma_start(y_d[:], yb[last][:])

    nc.compile()
    _DEV_CACHE[key] = nc
    return nc


def _host_tables(w_conv, b_conv, w_ct, b_ct, n_iters=N_ITERS, unroll=UNROLL):
    """Device weight tables (sample-independent)."""
    import ml_dtypes
    w_rev = w_ct[::-1, ::-1]                      # [a,b,ci,co]
    Wr = np.transpose(w_rev, (2, 0, 1, 3)).reshape(C, F).astype(np.float32)
    aa, bb_, cc = np.meshgrid(np.arange(5), np.arange(5), np.arange(C),
                              indexing='ij')
    Wgb = np.zeros((F, 25 * F), np.float32)
    for s in range(25):
        m, n = divmod(s, 5)
        Wcs = w_conv[8 * m + aa, 8 * n + bb_, cc, :].reshape(F, C)
        Wgb[:, F * s:F * (s + 1)] = (
            Wcs.astype(np.float64) @ Wr.astype(np.float64)).astype(np.float32)
    Wgb = Wgb.astype(ml_dtypes.bfloat16)
    Wa = np.zeros((EXT, F), np.float32)
    Wa[0:F] = np.eye(F, dtype=np.float32)
    Wa[F:F + 3] = Wr                               # bx rows: Wr(b_conv - x)
    Wa[F + 3] = np.broadcast_to(-b_ct, (5, 5, C)).reshape(F)

    betas = _betas(max(n_iters + 1, 1))
    sched = np.zeros((128, 8 * SBG), np.float32)
    for n in range(n_iters):
        k, i = n % unroll, n // unroll
        sched[:, k * SBG + i] = np.float32(1.0) + betas[n]   # s = 1 + beta
        sched[:, (unroll + k) * SBG + i] = betas[n + 1]      # next beta (q)
    return Wa, Wgb, w_rev, sched


def kernel(x, lam, w_conv, b_conv, w_ct, b_ct):
    from concourse import bass_utils

    x = np.asarray(x, np.float32)
    lam = np.asarray(lam, np.float32)
    w_conv = np.asarray(w_conv, np.float32)
    b_conv = np.asarray(b_conv, np.float32)
    w_ct = np.asarray(w_ct, np.float32)
    b_ct = np.asarray(b_ct, np.float32)
    B = x.shape[0]

    Wa, Wgb, w_rev, sched = _host_tables(w_conv, b_conv, w_ct, b_ct)

    # ---- host analysis (exact): c = At(x - b_conv) + b_ct on the live grid
    xb = x - b_conv                               # (B,9,9,3)
    c = np.einsum('abeo,sije->sabo' 'ij', w_rev, xb, optimize=True)
    c = c + b_ct[None, None, None, :, None, None]
    cmax = np.abs(c).max(axis=(1, 2, 3, 4, 5))
    active = cmax > lam * np.float32(1.0 - 1e-5)
    act_idx = np.where(active)[0]

    out = np.zeros((B, HW, HW, C), np.float32)

    # Non-patch positions evolve autonomously: w = y - b_ct per channel.
    # Exact when b_ct == 0 (it is, per the model); otherwise computed here.
    if np.any(b_ct != 0.0):
        betas = _betas(N_ITERS)
        yv = np.zeros((B, C), np.float32)
        yl = np.zeros((B, C), np.float32)
        for n in range(N_ITERS):
            w_np = yv - b_ct[None, :]
            y_new = (np.maximum(w_np - lam[:, None], 0)
                     - np.maximum(-w_np - lam[:, None], 0)).astype(np.float32)
            yv = y_new + np.float32(betas[n]) * (y_new - yl)
            yl = y_new
        mask = np.ones((HW, HW), bool)
        rows = (np.arange(HW) % 8) < 5
        mask[np.ix_(rows, rows)] = False          # live-grid positions
        out[:, mask, :] = yl[:, None, :]

    nc = _build_device(N_ITERS)

    n_rounds = max(1, math.ceil(len(act_idx) / N_CORES))
    zero_bxp = np.zeros((4, NPOS), np.float32)
    zero_bxp[3, :] = 1.0
    one_lam = np.stack([np.ones(F, np.float32), -np.ones(F, np.float32)],
                       axis=1)
    for r in range(n_rounds):
        batch = act_idx[r * N_CORES:(r + 1) * N_CORES]
        in_maps = []
        for k in range(N_CORES):
            if k < len(batch):
                s = int(batch[k])
                bxp = np.zeros((4, NPOS), np.float32)
                bxp[0:3] = b_conv[:, None] - x[s].reshape(NPOS, C).T
                bxp[3, :] = 1.0
                lam2 = np.stack([np.full(F, lam[s], np.float32),
                                 np.full(F, -lam[s], np.float32)], axis=1)
            else:
                bxp, lam2 = zero_bxp, one_lam
            in_maps.append({
                "wa": Wa, "wgb": Wgb, "bxp": np.ascontiguousarray(bxp),
                "lam2": np.ascontiguousarray(lam2), "sched": sched,
            })
        res = bass_utils.run_bass_kernel_spmd(nc, in_maps,
                                              core_ids=list(range(N_CORES)))
        for k in range(len(batch)):
            s = int(batch[k])
            ya = np.asarray(res.results[k]["y"], np.float32
                            ).reshape(5, 5, C, NB, NB)
            # out[s, 8I+a, 8J+b, co] = ya[a,b,co,I,J]
            blk = np.transpose(ya, (3, 0, 4, 1, 2))   # (I,a,J,b,co)
            ov = out[s].reshape(NB, 8, NB, 8, C)
            ov[:, :5, :, :5, :] = blk
    return out
